# revision 8
# baseline (speedup 1.0000x reference)
"""HMM forward-backward (scaled) -> posterior gammas, on 8 Trainium2 cores.

Strategy
--------
gammas[t] = normalize(alpha_t * beta_t) is invariant to any per-timestep or
per-chain rescaling of alpha/beta.  A's Hilbert-metric contraction per step is
|lambda_2(A)| ~ 0.026, so a chain forgets its initial condition to below f32
resolution in ~5 steps.  We therefore split the 16384-step scan into
8 cores x B chains of L steps, each chain warming up for W steps from the
stationary vector pi of A -- zero cross-core communication.

Per core the recurrence runs as a batched matvec in "column" layout:
state^T tiles [128_state x B_chains], A-tiles as stationary weights,
16 matmuls + 4 vector multiplies per step.  Backward runs the substituted
variable d_t = beta_t * obs_t, which obeys the same-shaped recurrence
d_t = (A @ d_{t+1}) * obs_t with weights A^T; beta_t is the pre-multiply
matmul result.  Output slabs are converted to row layout (PE transposes)
so the per-timestep normalization can use free-dim reduces.

Exactness at the sequence edges:
  - forward chain 0: seed pi, warmup obs rows = ones (pi is a fixed point),
    and the final warmup row is pi0*obs[0]/pi which lands the chain exactly
    on alpha_0 (pi0 is uniform).
  - backward last chain: seed ones, virtual obs rows (t >= N) = ones; since
    A @ 1 = 1 the first real output is beta_{N-1} proportional to ones, exact.
"""

import numpy as np
import os
DBG = int(os.environ.get("KDBG", "0"))

N = 16384
S = 512
NCORES = 8
TCORE = N // NCORES          # 2048 timesteps per core
B = 128                      # chains per core (= SBUF partitions)
L = TCORE // B               # 16 outputs per chain
W = 6                        # warmup steps
ST = L + W                   # machinery steps per direction
KB = 4                       # state blocks of 128 (S = 4*128)
PRESCALE = 2.0               # keeps per-step magnitude drift ~1, no rescaling
USE_FP32R = False             # PE fast path (1 cyc/row vs 4); checked vs f64 ref

_BUILT = None                # cached (nc, names)


# ----------------------------------------------------------------- host prep
def _host_prep(obs, A, pi0):
    """Build per-core input arrays (all f32, layouts ready for straight DMA)."""
    A64 = A.astype(np.float64)
    # stationary left eigenvector of A
    pi = np.full(S, 1.0 / S)
    for _ in range(60):
        pi = pi @ A64
    pi /= pi.sum()

    obs2 = (obs.astype(np.float64) * PRESCALE).astype(np.float32)
    crafted0 = ((pi0.astype(np.float64) * obs2[0].astype(np.float64)) / pi
                ).astype(np.float32)

    def col_layout(rows):  # [ST, B, S] -> [ST, 128, KB*B]
        return np.ascontiguousarray(
            rows.reshape(ST, B, KB, 128).transpose(0, 3, 2, 1).reshape(ST, 128, KB * B))

    s_idx = np.arange(ST)[:, None]
    b_idx = np.arange(B)[None, :]

    obsf, obsb = [], []
    for c in range(NCORES):
        t0 = c * TCORE
        # forward: step s of chain b consumes obs[t0 + b*L - W + s]
        tf = t0 + b_idx * L - W + s_idx
        rows = obs2[np.clip(tf, 0, N - 1)].copy()
        if c == 0:
            rows[:W, 0] = 1.0          # chain 0 warmup: ones (pi fixed point)
            rows[W, 0] = crafted0      # lands exactly on alpha_0
        obsf.append(col_layout(rows))
        # backward (d-form): step s of chain b consumes obs[t] at its OUTPUT t
        tb = t0 + b_idx * L + (L - 1) + W - s_idx
        rows = np.where((tb <= N - 1)[:, :, None],
                        obs2[np.clip(tb, 0, N - 1)], np.float32(1.0))
        obsb.append(np.ascontiguousarray(col_layout(rows)))

    a_t = np.ascontiguousarray(
        A.reshape(KB, 128, S).transpose(1, 0, 2).reshape(128, KB * S)).astype(np.float32)
    at_t = np.ascontiguousarray(
        A.T.reshape(KB, 128, S).transpose(1, 0, 2).reshape(128, KB * S)).astype(np.float32)

    seedf = np.ascontiguousarray(
        np.broadcast_to(pi.astype(np.float32).reshape(KB, 128).T[:, :, None],
                        (128, KB, B)).reshape(128, KB * B))
    ident = np.eye(128, dtype=np.float32)

    seedb = np.ones((128, KB * B), np.float32)
    return obsf, obsb, a_t, at_t, seedf, seedb, ident


# -------------------------------------------------------------- kernel build
def _build():
    global _BUILT
    if _BUILT is not None:
        return _BUILT

    import concourse.bass as bass
    import concourse.tile as tile
    import concourse.mybir as mybir
    from concourse import bacc

    f32 = mybir.dt.float32
    mmdt = mybir.dt.float32r if USE_FP32R else f32
    MULT = mybir.AluOpType.mult
    ADD = mybir.AluOpType.add
    COPY = mybir.ActivationFunctionType.Copy

    DT = mmdt  # scan-path dtype (fp32r tensors are produced rounded)

    nc = bacc.Bacc("TRN2", target_bir_lowering=False, debug=False)

    obsf_d = nc.dram_tensor("obsf", [ST, 128, KB * B], mmdt, kind="ExternalInput")
    obsb_d = nc.dram_tensor("obsb", [ST, 128, KB * B], mmdt, kind="ExternalInput")
    a_d = nc.dram_tensor("a_t", [128, KB * S], mmdt, kind="ExternalInput")
    at_d = nc.dram_tensor("at_t", [128, KB * S], mmdt, kind="ExternalInput")
    seedf_d = nc.dram_tensor("seedf", [128, KB * B], mmdt, kind="ExternalInput")
    seedb_d = nc.dram_tensor("seedb", [128, KB * B], mmdt, kind="ExternalInput")
    ident_d = nc.dram_tensor("ident", [128, 128], mmdt, kind="ExternalInput")
    gout_d = nc.dram_tensor("gout", [TCORE, S], f32, kind="ExternalOutput")
    gout_r = gout_d[:].rearrange("(b l) s -> b l s", l=L)

    with tile.TileContext(nc) as tc:
        with (
            tc.tile_pool(name="const", bufs=1) as const_pool,
            tc.tile_pool(name="store", bufs=1) as store_pool,
            tc.tile_pool(name="obsp", bufs=3) as obs_pool,
            tc.tile_pool(name="state", bufs=2) as st_pool,
            tc.tile_pool(name="betacp", bufs=2) as bcp_pool,
            tc.tile_pool(name="mm", bufs=1, space="PSUM") as mm_pool,
            tc.tile_pool(name="trp", bufs=2, space="PSUM") as tr_pool,
            tc.tile_pool(name="gam", bufs=3) as gam_pool,
        ):
            # constants
            a_sb = const_pool.tile([128, KB * S], mmdt, name="a_sb")
            at_sb = const_pool.tile([128, KB * S], mmdt, name="at_sb")
            ident_sb = const_pool.tile([128, 128], mmdt, name="ident_sb")
            seedf_sb = const_pool.tile([128, KB * B], mmdt, name="seedf_sb")
            nc.sync.dma_start(a_sb[:], a_d[:])
            nc.sync.dma_start(at_sb[:], at_d[:])
            nc.sync.dma_start(ident_sb[:], ident_d[:])
            nc.sync.dma_start(seedf_sb[:], seedf_d[:])
            seedb_sb = const_pool.tile([128, KB * B], mmdt, name="seedb_sb")
            nc.sync.dma_start(seedb_sb[:], seedb_d[:])

            # row-layout output stores (slab j holds t = b*L + j on partition b)
            alpha_row = store_pool.tile([128, L * S], f32, name="alpha_row")
            beta_row = store_pool.tile([128, L * S], f32, name="beta_row")

            prev_f = seedf_sb
            prev_b = seedb_sb
            for s in range(ST):
                out_step = (s >= W) and DBG < 2
                # ---------------- forward: state' = (state @ A) * obs ------
                of = obs_pool.tile([128, KB * B], mmdt, name="of", tag="of")
                nc.sync.dma_start(of[:], obsf_d[s])
                cur_f = st_pool.tile([128, KB * B], mmdt, name="stf", tag="stf")
                pf = [mm_pool.tile([128, 2 * B], f32, name="pf01", tag="pf01"),
                      mm_pool.tile([128, 2 * B], f32, name="pf23", tag="pf23")]
                for m in range(KB):
                    pt = pf[m // 2]
                    col = (m % 2) * B
                    for k in range(KB):
                        nc.tensor.matmul(
                            pt[:, col:col + B],
                            a_sb[:, k * S + m * 128: k * S + (m + 1) * 128],
                            prev_f[:, k * B:(k + 1) * B],
                            start=(k == 0), stop=(k == KB - 1))
                    nc.vector.tensor_tensor(
                        cur_f[:, m * B:(m + 1) * B], pt[:, col:col + B],
                        of[:, m * B:(m + 1) * B], MULT)
                if out_step:
                    j = s - W
                    trf = tr_pool.tile([128, 512], mmdt, name="trf", tag="tr")
                    for k in range(KB):
                        nc.tensor.transpose(
                            trf[:, k * 128:(k + 1) * 128],
                            cur_f[:, k * B:(k + 1) * B], ident_sb[:])
                        nc.scalar.activation(
                            alpha_row[:, j * S + k * 128: j * S + (k + 1) * 128],
                            trf[:, k * 128:(k + 1) * 128], COPY)
                prev_f = cur_f

                # ------- backward (d-form): beta = A @ d ; d' = beta * obs --
                last = s == ST - 1
                ob = obs_pool.tile([128, KB * B], mmdt, name="ob", tag="ob")
                nc.sync.dma_start(ob[:], obsb_d[s])
                cur_b = None if last else st_pool.tile([128, KB * B], mmdt, name="stb", tag="stb")
                pb = [mm_pool.tile([128, 2 * B], f32, name="pb01", tag="pb01"),
                      mm_pool.tile([128, 2 * B], f32, name="pb23", tag="pb23")]
                if out_step:
                    bcp = bcp_pool.tile([128, KB * B], mmdt, name="bcp", tag="bcp")
                for m in range(KB):
                    pt = pb[m // 2]
                    col = (m % 2) * B
                    for k in range(KB):
                        nc.tensor.matmul(
                            pt[:, col:col + B],
                            at_sb[:, k * S + m * 128: k * S + (m + 1) * 128],
                            prev_b[:, k * B:(k + 1) * B],
                            start=(k == 0), stop=(k == KB - 1))
                    if out_step:
                        # beta_t = psum (pre-multiply); stash to SBUF via ACT
                        nc.scalar.activation(
                            bcp[:, m * B:(m + 1) * B], pt[:, col:col + B], COPY)
                    if not last:
                        nc.vector.tensor_tensor(
                            cur_b[:, m * B:(m + 1) * B], pt[:, col:col + B],
                            ob[:, m * B:(m + 1) * B], MULT)
                if out_step:
                    j = L - 1 - (s - W)
                    trb = tr_pool.tile([128, 512], mmdt, name="trb", tag="tr")
                    for k in range(KB):
                        nc.tensor.transpose(
                            trb[:, k * 128:(k + 1) * 128],
                            bcp[:, k * B:(k + 1) * B], ident_sb[:])
                        nc.scalar.activation(
                            beta_row[:, j * S + k * 128: j * S + (k + 1) * 128],
                            trb[:, k * 128:(k + 1) * 128], COPY)
                if not last:
                    prev_b = cur_b

            # ---------------- gammas: normalize(alpha * beta) per row ------
            if DBG >= 1:
                fin = gam_pool.tile([128, S], f32, name="fin", tag="gam")
                nc.vector.tensor_tensor(fin[:], prev_f[:].bitcast(f32), prev_b[:].bitcast(f32), MULT)
                nc.sync.dma_start(gout_r[:, 0, :], fin[:])
            for j in range(L if DBG == 0 else 0):
                joint = gam_pool.tile([128, S], f32, name="joint", tag="joint")
                rs = gam_pool.tile([128, 1], f32, name="rs", tag="rs")
                rc = gam_pool.tile([128, 1], f32, name="rc", tag="rc")
                gam = gam_pool.tile([128, S], f32, name="gam", tag="gam")
                nc.vector.tensor_tensor(
                    joint[:], alpha_row[:, j * S:(j + 1) * S],
                    beta_row[:, j * S:(j + 1) * S], MULT)
                nc.vector.reduce_sum(rs[:], joint[:], axis=mybir.AxisListType.X)
                nc.vector.reciprocal(rc[:], rs[:])
                nc.scalar.activation(gam[:], joint[:], COPY, scale=rc[:])
                nc.sync.dma_start(gout_r[:, j, :], gam[:])

    nc.compile()
    _BUILT = nc
    return nc


# ------------------------------------------------------------------- driver
def kernel(observation_probs, A, pi0):
    obs = np.asarray(observation_probs, dtype=np.float32)
    A = np.asarray(A, dtype=np.float32)
    pi0 = np.asarray(pi0, dtype=np.float32)

    obsf, obsb, a_t, at_t, seedf, seedb, ident = _host_prep(obs, A, pi0)
    nc = _build()

    from concourse.bass_utils import run_bass_kernel_spmd
    in_maps = [{
        "obsf": obsf[c], "obsb": obsb[c],
        "a_t": a_t, "at_t": at_t, "seedf": seedf, "seedb": seedb, "ident": ident,
    } for c in range(NCORES)]
    res = run_bass_kernel_spmd(nc, in_maps, core_ids=list(range(NCORES)))
    out = np.concatenate([res.results[c]["gout"] for c in range(NCORES)], axis=0)
    return out.astype(np.float32)
